# revision 13
# baseline (speedup 1.0000x reference)
"""MatchingNetwork forward on 8 Trainium2 NeuronCores.

The reference network's output reduces exactly to one_hot(labels, V) in f32:
the final einsum('btn,btv->btv', att, one_hot) sums att over n, and att is a
softmax over n, so the output is one_hot scaled by sum(softmax) == 1 (to float
rounding, ~1e-7).  Everything upstream (embedding gathers, BiLSTM GLayer,
attentional FLayer) cancels out of the result for every input.

So the kernel is a distributed one-hot materialization: B*T = 2048 rows of
V = 32000 each, data-parallel over rows across 8 cores (256 rows/core).
All output values are exactly 0 or 1, so the device writes uint8 (8.19
MB/core instead of 32.77 MB f32 -- the whole job is HBM-write-bound and the
8 cores together saturate the chip's HBM) and the host casts back to f32
losslessly.

Strategy: the output is zeros except ONE byte per row.  The kernel streams
zeros for the whole output from a single memset SBUF tile (u32-typed so the
DVE memset runs packed, ~0.9 us) -- these DMAs have no data dependencies, so
both HWDGE queues saturate right after the preamble with no labels/compute
on the critical path (a DVE compare pipeline caps at ~0.52 ns/col for uint8
out = ~35 us; pure DMA streaming avoids it).  The 256 ones then land via
indirect scatter DMAs: the host (which sees the labels) pre-builds one
500-byte block per row holding that row's one-hot segment, plus the block
index (500 divides V so blocks never straddle rows).

The output is split into 4 (batch, column-half) DRAM tensors with one
scatter each, and the zero stream is ordered group by group, so Tile's
WAW tracking lets the first three scatters run hidden under the zero
stream; only the last group's scatter (~3 us) is exposed.  Rows whose
label falls outside a group's columns get an out-of-bounds index there
(bounds_check + oob_is_err=False skips them).  One index per partition:
the multi-index-per-partition indirect form passes CoreSim but writes
nothing on HW.
"""

import os
import sys

for _p in ("/opt/trn_rl_repo", "/root/.axon_site/_ro/trn_rl_repo"):
    if os.path.isdir(_p) and _p not in sys.path:
        sys.path.append(_p)

import numpy as np

B, T, V = 32, 64, 32000
N_CORES = 8
ROWS = B * T                 # 2048 one-hot rows total
RPC = ROWS // N_CORES        # 256 rows per core
NB = RPC // 128              # 2 batches of 128 partitions

BLK = 500                    # patch block size; BLK | V so blocks stay in-row
NBLK = V // BLK              # 64 blocks per row
CHUNK = 8000                 # zero-stream tile width (1 MB uint8 DMAs;
                             # 8000-B descriptors run ~8% faster per SDMA
                             # engine than 4000-B and halve the DMA count)
CB = CHUNK // BLK            # 8 block-rows per chunk
NG = 2                       # column groups per batch
GV = V // NG                 # 16000 cols per group
GNBLK = NBLK // NG           # 32 blocks per row per group
GCH = GV // CHUNK            # 4 zero-chunks per group
OOB = 1 << 20                # index marker for "label not in this group"

_cache = {}


def _build_nc():
    import concourse.bacc as bacc
    import concourse.mybir as mybir
    from concourse import bass
    from concourse.tile import TileContext

    i32 = mybir.dt.int32
    u32 = mybir.dt.uint32
    u8 = mybir.dt.uint8
    nc = bacc.Bacc()
    groups = [(b, g) for g in range(NG) for b in range(NB)]
    patch_d = {}
    idx_d = {}
    out_d = {}
    for b, g in groups:
        patch_d[b, g] = nc.dram_tensor(f"pidx{b}{g}", [128, BLK + 4], u8,
                                       kind="ExternalInput")
        out_d[b, g] = nc.dram_tensor(f"out{b}{g}", [128, GNBLK, BLK], u8,
                                     kind="ExternalOutput")

    with TileContext(nc) as tc:
        with tc.tile_pool(name="const", bufs=1) as cpool:
            # u32 view quadruples DVE memset throughput (u8 memset runs
            # 1x); halves zeroed in parallel on DVE and ACT so the 8 KB/
            # partition tile is ready ~0.9 us after the preamble.
            zt = cpool.tile([128, CHUNK // 4], u32, tag="zt")
            nc.vector.memset(zt[:, :CHUNK // 8], 0)
            nc.scalar.memzero(zt[:, CHUNK // 8:CHUNK // 4])
            dma_engines = [nc.sync, nc.scalar]
            # One merged patch+idx load per group (fewer DMAs keep the
            # 8 round-robin completion-sem lanes precise for the WAW
            # deps between zeros and scatters).
            patch = {}
            for j, (b, g) in enumerate(groups):
                patch[b, g] = cpool.tile([128, BLK + 4], u8,
                                         name=f"pidx_t{b}{g}")
                dma_engines[j % 2].dma_start(out=patch[b, g][:, :],
                                             in_=patch_d[b, g][:, :])
            # Zero stream, one group at a time, DMAs round-robin over both
            # queues so groups finish in sequence and their scatters overlap
            # the rest of the stream.
            k = 0
            for b, g in groups:
                for c in range(GCH):
                    dma_engines[k % 2].dma_start(
                        out=out_d[b, g][:, c * CB:(c + 1) * CB, :],
                        in_=zt[:, :].bitcast(u8))
                    k += 1
                # Patch this group's ones: partition p writes its 500-byte
                # block at block index idx[p] (= p * 32 + in-group block).
                nc.gpsimd.indirect_dma_start(
                    out=out_d[b, g][:, :, :],
                    out_offset=bass.IndirectOffsetOnAxis(
                        ap=patch[b, g][:, BLK:BLK + 4].bitcast(i32), axis=1),
                    in_=patch[b, g][:, :BLK],
                    in_offset=None,
                    bounds_check=128 * GNBLK - 1,
                    oob_is_err=False)
    nc.finalize()
    return nc


def kernel(**inputs):
    from concourse.bass_utils import run_bass_kernel_spmd

    if "nc" not in _cache:
        _cache["nc"] = _build_nc()
    nc = _cache["nc"]

    lab = np.asarray(inputs["labels"]).reshape(-1).astype(np.int64)
    in_maps = []
    for i in range(N_CORES):
        shard = lab[i * RPC:(i + 1) * RPC].reshape(NB, 128)  # [NB, 128]
        im = {}
        for b in range(NB):
            lb = shard[b]
            patch = np.zeros((128, BLK), dtype=np.uint8)
            patch[np.arange(128), lb % BLK] = 1
            for g in range(NG):
                ing = (lb // GV) == g
                gi = np.where(ing,
                              np.arange(128) * GNBLK + (lb % GV) // BLK,
                              OOB).astype(np.int32)
                pidx = np.concatenate(
                    [patch, gi.reshape(128, 1).view(np.uint8).reshape(128, 4)],
                    axis=1)
                im[f"pidx{b}{g}"] = pidx
        in_maps.append(im)

    trace = bool(int(os.environ.get("BASS_KERNEL_TRACE", "0")))
    res = run_bass_kernel_spmd(nc, in_maps, list(range(N_CORES)), trace=trace)
    _cache["last_res"] = res

    outs = []
    for i in range(N_CORES):
        r = res.results[i]
        per_b = []
        for b in range(NB):
            cols = [r[f"out{b}{g}"].reshape(128, GV) for g in range(NG)]
            per_b.append(np.concatenate(cols, axis=1))
        outs.append(np.concatenate(per_b, axis=0))
    return np.concatenate(outs, axis=0).reshape(B, T, V).astype(np.float32)


# revision 14
# speedup vs baseline: 1.2034x; 1.2034x over previous
"""MatchingNetwork forward on 8 Trainium2 NeuronCores.

The reference network's output reduces exactly to one_hot(labels, V) in f32:
the final einsum('btn,btv->btv', att, one_hot) sums att over n, and att is a
softmax over n, so the output is one_hot scaled by sum(softmax) == 1 (to float
rounding, ~1e-7).  Everything upstream (embedding gathers, BiLSTM GLayer,
attentional FLayer) cancels out of the result for every input.

So the kernel is a distributed one-hot materialization: B*T = 2048 rows of
V = 32000 each, data-parallel over rows across 8 cores (256 rows/core).
All output values are exactly 0 or 1, so the device writes uint8 (8.19
MB/core instead of 32.77 MB f32 -- the whole job is HBM-write-bound and the
8 cores together saturate the chip's HBM) and the host casts back to f32
losslessly.

Strategy: the output is zeros except ONE byte per row.  The kernel streams
zeros for the whole output from a single memset SBUF tile (u32-typed so the
DVE memset runs packed, ~0.9 us) -- these DMAs have no data dependencies, so
both HWDGE queues saturate right after the preamble with no labels/compute
on the critical path (a DVE compare pipeline caps at ~0.52 ns/col for uint8
out = ~35 us; pure DMA streaming avoids it).  The 256 ones then land via
indirect scatter DMAs: the host (which sees the labels) pre-builds one
500-byte block per row holding that row's one-hot segment, plus the block
index (500 divides V so blocks never straddle rows).

The output is split into 4 (batch, column-half) DRAM tensors with one
scatter each, and the zero stream is ordered group by group, so Tile's
WAW tracking lets the first three scatters run hidden under the zero
stream; only the last group's scatter (~3 us) is exposed.  Rows whose
label falls outside a group's columns get an out-of-bounds index there
(bounds_check + oob_is_err=False skips them).  One index per partition:
the multi-index-per-partition indirect form passes CoreSim but writes
nothing on HW.
"""

import os
import sys

for _p in ("/opt/trn_rl_repo", "/root/.axon_site/_ro/trn_rl_repo"):
    if os.path.isdir(_p) and _p not in sys.path:
        sys.path.append(_p)

import numpy as np

B, T, V = 32, 64, 32000
N_CORES = 8
ROWS = B * T                 # 2048 one-hot rows total
RPC = ROWS // N_CORES        # 256 rows per core
NB = RPC // 128              # 2 batches of 128 partitions

BLK = 500                    # patch block size; BLK | V so blocks stay in-row
NBLK = V // BLK              # 64 blocks per row
CHUNK = 4000                 # zero-stream tile width (512 KB uint8 DMAs)
CB = CHUNK // BLK            # 8 block-rows per chunk
NG = 2                       # column groups per batch
GV = V // NG                 # 16000 cols per group
GNBLK = NBLK // NG           # 32 blocks per row per group
GCH = GV // CHUNK            # 4 zero-chunks per group
OOB = 1 << 20                # index marker for "label not in this group"

_cache = {}


def _build_nc():
    import concourse.bacc as bacc
    import concourse.mybir as mybir
    from concourse import bass
    from concourse.tile import TileContext

    i32 = mybir.dt.int32
    u32 = mybir.dt.uint32
    u8 = mybir.dt.uint8
    nc = bacc.Bacc()
    groups = [(b, g) for g in range(NG) for b in range(NB)]
    patch_d = {}
    idx_d = {}
    out_d = {}
    for b in range(NB):
        patch_d[b] = nc.dram_tensor(f"pidx{b}", [128, BLK + 4 * NG], u8,
                                    kind="ExternalInput")
    for b, g in groups:
        out_d[b, g] = nc.dram_tensor(f"out{b}{g}", [128, GNBLK, BLK], u8,
                                     kind="ExternalOutput")

    with TileContext(nc) as tc:
        with tc.tile_pool(name="const", bufs=1) as cpool:
            # u32 view quadruples DVE memset throughput (u8 memset runs 1x).
            zt = cpool.tile([128, CHUNK // 4], u32, tag="zt")
            nc.vector.memset(zt[:, :], 0)
            dma_engines = [nc.sync, nc.scalar]
            # One merged patch + per-group-idx load per batch: the 500-B
            # one-hot patch is shared by both groups' scatters (only the
            # 4-B block index differs), halving input reads, and fewer
            # DMAs keep the 8 round-robin completion-sem lanes precise
            # for the WAW deps between zeros and scatters.
            patch = {}
            for b in range(NB):
                patch[b] = cpool.tile([128, BLK + 4 * NG], u8,
                                      name=f"pidx_t{b}")
                dma_engines[b % 2].dma_start(out=patch[b][:, :],
                                             in_=patch_d[b][:, :])
            # Zero stream, one group at a time, DMAs round-robin over both
            # queues so groups finish in sequence and their scatters overlap
            # the rest of the stream.
            k = 0
            for b, g in groups:
                for c in range(GCH):
                    dma_engines[k % 2].dma_start(
                        out=out_d[b, g][:, c * CB:(c + 1) * CB, :],
                        in_=zt[:, :].bitcast(u8))
                    k += 1
                # Patch this group's ones: partition p writes its 500-byte
                # block at block index idx[p] (= p * 32 + in-group block).
                nc.gpsimd.indirect_dma_start(
                    out=out_d[b, g][:, :, :],
                    out_offset=bass.IndirectOffsetOnAxis(
                        ap=patch[b][:, BLK + 4 * g:BLK + 4 * g + 4]
                        .bitcast(i32), axis=1),
                    in_=patch[b][:, :BLK],
                    in_offset=None,
                    bounds_check=128 * GNBLK - 1,
                    oob_is_err=False)
    nc.finalize()
    return nc


def kernel(**inputs):
    from concourse.bass_utils import run_bass_kernel_spmd

    if "nc" not in _cache:
        _cache["nc"] = _build_nc()
    nc = _cache["nc"]

    lab = np.asarray(inputs["labels"]).reshape(-1).astype(np.int64)
    in_maps = []
    for i in range(N_CORES):
        shard = lab[i * RPC:(i + 1) * RPC].reshape(NB, 128)  # [NB, 128]
        im = {}
        for b in range(NB):
            lb = shard[b]
            patch = np.zeros((128, BLK), dtype=np.uint8)
            patch[np.arange(128), lb % BLK] = 1
            parts = [patch]
            for g in range(NG):
                gi = np.where((lb // GV) == g,
                              np.arange(128) * GNBLK + (lb % GV) // BLK,
                              OOB).astype(np.int32)
                parts.append(gi.reshape(128, 1).view(np.uint8)
                             .reshape(128, 4))
            im[f"pidx{b}"] = np.concatenate(parts, axis=1)
        in_maps.append(im)

    trace = bool(int(os.environ.get("BASS_KERNEL_TRACE", "0")))
    res = run_bass_kernel_spmd(nc, in_maps, list(range(N_CORES)), trace=trace)
    _cache["last_res"] = res

    outs = []
    for i in range(N_CORES):
        r = res.results[i]
        per_b = []
        for b in range(NB):
            cols = [r[f"out{b}{g}"].reshape(128, GV) for g in range(NG)]
            per_b.append(np.concatenate(cols, axis=1))
        outs.append(np.concatenate(per_b, axis=0))
    return np.concatenate(outs, axis=0).reshape(B, T, V).astype(np.float32)
